# revision 2
# baseline (speedup 1.0000x reference)
"""Trainium2 Bass kernel for nn_AverageAttention (B=8, L=2048, D=1024).

Math (per batch b):
    avg[t]  = cumsum(x, axis=t)[t] / (t+1)
    g       = concat([x, avg], -1) @ W_gate.T + b_gate        # (L, 2*D)
    out     = sigmoid(g[:, :D]) * x + sigmoid(g[:, D:]) * avg

Strategy: batch-parallel over 8 NeuronCores (one sequence per core), W_gate
replicated. On-chip layout is transposed (feature-on-partition,
token-on-free) so the cumulative sum is a single DVE tensor_tensor_scan per
128-feature chunk. The gating matmul runs in fp8_e4m3 with
perf_mode=DoubleRow (two 128-row contraction chunks per instruction, ~1.4x
bf16 PE throughput); operands are pre-scaled (x*16, avg*16, W*64) to keep
fp8 out of the subnormal range, and the 1/1024 descale plus bias fold into
the sigmoid PSUM-evacuation on the scalar engine.
"""

from contextlib import ExitStack

import ml_dtypes
import numpy as np

import concourse.bass as bass
import concourse.bass_utils as bass_utils
import concourse.mybir as mybir
import concourse.tile as tile
from concourse import bacc
from concourse._compat import with_exitstack
from concourse.bass import ts

B, L, D = 8, 2048, 1024
NJ = D // 128        # 8 feature chunks of x / avg (= DoubleRow pairs)
NOB = 2 * D // 128   # 16 output-feature blocks of g
TCW = 512            # matmul moving free-dim (DoubleRow max: 2*512)
NTC = L // TCW       # token chunks

# fp8 pre-scales. S = AX*BW = AA*BW must be uniform across both halves so
# the PSUM accumulation is uniformly scaled; sigmoid evacuation applies 1/S.
AX = 16.0            # x -> fp8 scale
AA = 16.0            # avg -> fp8 scale
BW = 64.0            # W -> fp8 scale
SINV = 1.0 / (AX * BW)

FP32 = mybir.dt.float32
BF16 = mybir.dt.bfloat16
FP8 = mybir.dt.float8e4

import os as _os_mod


@with_exitstack
def _tile_body(
    ctx: ExitStack,
    tc: tile.TileContext,
    reps: int = 1,
    gt_inplace: bool = True,
):
    nc = tc.nc

    xT = nc.dram_tensor("xT", (NJ, 128, L), FP32, kind="ExternalInput").ap()
    wdr = nc.dram_tensor("wdr", (NOB, 128, NJ, 2, 128), FP8, kind="ExternalInput").ap()
    invd = nc.dram_tensor("invd", (128, L), FP32, kind="ExternalInput").ap()
    biash = nc.dram_tensor("biash", (128, NOB), FP32, kind="ExternalInput").ap()
    avgT = nc.dram_tensor("avgT", (NJ, 128, L), FP32, kind="ExternalOutput").ap()
    gatT = nc.dram_tensor("gatT", (NJ, 128, L), FP32, kind="ExternalOutput").ap()

    const_pool = ctx.enter_context(tc.tile_pool(name="const", bufs=1))
    x_pool = ctx.enter_context(tc.tile_pool(name="x", bufs=NJ))
    at_pool = ctx.enter_context(tc.tile_pool(name="at", bufs=2))
    abf_pool = ctx.enter_context(tc.tile_pool(name="abf", bufs=NJ))
    cat_pool = ctx.enter_context(tc.tile_pool(name="cat", bufs=NJ))
    ct_pool = ctx.enter_context(tc.tile_pool(name="ct", bufs=1))
    w_pool = ctx.enter_context(tc.tile_pool(name="w", bufs=3))
    sig_pool = ctx.enter_context(tc.tile_pool(name="sig", bufs=3))
    psum_pool = ctx.enter_context(tc.tile_pool(name="psum", bufs=8, space="PSUM"))
    if not gt_inplace:
        gat_pool = ctx.enter_context(tc.tile_pool(name="gat", bufs=1))

    invd_sb = const_pool.tile([128, L], FP32, tag="invd")
    bias_sb = const_pool.tile([128, NOB], FP32, tag="bias")

    for _rep in range(reps):
        # cat[j][:, 0, :] = fp8(AX*x_j); cat[j][:, 1, :] = fp8(AA*avg_j).
        cats = [
            cat_pool.tile([128, 2, L], FP8, tag="cat", name=f"cat{j}")
            for j in range(NJ)
        ]
        xts = [
            x_pool.tile([128, L], FP32, tag="xt", name=f"xt{j}") for j in range(NJ)
        ]
        abfs = [
            abf_pool.tile([128, L], BF16, tag="abf", name=f"abf{j}")
            for j in range(NJ)
        ]

        # Input DMA head ordering (FIFO per ring): x chunks stream on the
        # sync ring (phase-1 critical path); constants ride the scalar ring.
        if _rep == 0:
            nc.scalar.dma_start(bias_sb[:], biash[:])
            nc.scalar.dma_start(invd_sb[:], invd[:])

        # Phase 1: per chunk j: load x, cast fp8 x (gpsimd), cumsum scan +
        # 1/(t+1) scale (DVE), store avg, cast fp8 avg (scalar) and bf16 avg
        # (gpsimd, for the final combine).
        for j in range(NJ):
            nc.sync.dma_start(xts[j][:], xT[j])
        for j in range(NJ):
            xt = xts[j]
            ct = ct_pool.tile([128, L], FP32)
            at = at_pool.tile([128, L], FP32)
            nc.gpsimd.tensor_scalar_mul(cats[j][:, 0, :], xt[:], AX)
            nc.vector.tensor_tensor_scan(
                ct[:],
                xt[:],
                xt[:],
                0.0,
                mybir.AluOpType.add,
                mybir.AluOpType.bypass,
            )
            nc.vector.tensor_mul(at[:], ct[:], invd_sb[:])
            nc.scalar.dma_start(avgT[j], at[:])
            nc.scalar.activation(
                cats[j][:, 1, :],
                at[:],
                mybir.ActivationFunctionType.Copy,
                scale=AA,
            )
            nc.gpsimd.tensor_copy(abfs[j][:], at[:])

        # Phase 2: g^T blocks via fp8 DoubleRow matmul, weight-stationary
        # across the 4 token-chunk PSUM groups. Output blocks are visited in
        # (input_gate j, forget_gate j) pairs so each chunk's gate combine
        # runs (and its SBUF frees) as early as possible.
        def load_w(ob):
            wt = w_pool.tile([128, NJ, 2, 128], FP8, name=f"wt{ob}", tag="wt")
            nc.sync.dma_start(wt[:], wdr[ob])
            return wt

        OB_ORDER = []
        for j in range(NJ):
            OB_ORDER.extend([j, NJ + j])

        w_tiles = {ob: load_w(ob) for ob in OB_ORDER[:2]}
        sts = {}
        for n, ob in enumerate(OB_ORDER):
            if n + 2 < NOB:
                w_tiles[OB_ORDER[n + 2]] = load_w(OB_ORDER[n + 2])
            wt = w_tiles.pop(ob)
            st = sig_pool.tile([128, L], FP32, name=f"st{ob}", tag="st")
            pss = [
                psum_pool.tile([128, TCW], FP32, name="ps", tag="ps")
                for _ in range(NTC)
            ]
            for j in range(NJ):
                for tcx in range(NTC):
                    nc.tensor.matmul(
                        pss[tcx][:],
                        wt[:, j, :, :],
                        cats[j][:, :, ts(tcx, TCW)],
                        start=(j == 0),
                        stop=(j == NJ - 1),
                        perf_mode=mybir.MatmulPerfMode.DoubleRow,
                    )
            for tcx in range(NTC):
                nc.scalar.activation(
                    st[:, ts(tcx, TCW)],
                    pss[tcx][:],
                    mybir.ActivationFunctionType.Sigmoid,
                    bias=bias_sb[:, ob : ob + 1],
                    scale=SINV,
                )
            sts[ob] = st
            if ob >= NJ:
                # Both gates for chunk j ready: combine and store.
                j = ob - NJ
                st_i = sts.pop(j)
                st_f = sts.pop(ob)
                if gt_inplace:
                    nc.vector.tensor_mul(st_i[:], st_i[:], xts[j][:])
                    nc.vector.tensor_mul(st_f[:], st_f[:], abfs[j][:])
                    nc.vector.tensor_add(st_i[:], st_i[:], st_f[:])
                    nc.sync.dma_start(gatT[j], st_i[:])
                else:
                    gt = gat_pool.tile([128, L], FP32, name="gt", tag="gt")
                    nc.vector.tensor_mul(gt[:], st_i[:], xts[j][:])
                    nc.vector.tensor_mul(st_f[:], st_f[:], abfs[j][:])
                    nc.vector.tensor_add(gt[:], gt[:], st_f[:])
                    nc.sync.dma_start(gatT[j], gt[:])


_CACHE: dict = {}


def build_nc(reps: int | None = None, gt_inplace: bool | None = None):
    import os as _os

    if reps is None:
        reps = int(_os.environ.get("KREPS", "1"))
    if gt_inplace is None:
        gt_inplace = _os.environ.get("KGTIP", "1") == "1"
    key = ("nc", reps, gt_inplace)
    if key not in _CACHE:
        nc = bacc.Bacc(
            "TRN2",
            target_bir_lowering=False,
            debug=False,
            enable_asserts=True,
            num_devices=B,
        )
        with tile.TileContext(nc) as t:
            _tile_body(t, reps=reps, gt_inplace=gt_inplace)
        nc.compile()
        _CACHE[key] = nc
    return _CACHE[key]


def prep_shared(W_gate: np.ndarray, b_gate: np.ndarray):
    # wdr[ob, p, j, half, m] = BW * W_gate[128*ob + m, half*1024 + 128*j + p]
    wdr = np.ascontiguousarray(
        (W_gate.astype(np.float32) * BW)
        .reshape(NOB, 128, 2, NJ, 128)
        .transpose(0, 4, 3, 2, 1)
    ).astype(ml_dtypes.float8_e4m3)
    invd = np.ascontiguousarray(
        np.broadcast_to(
            1.0 / np.arange(1, L + 1, dtype=np.float32)[None, :], (128, L)
        )
    )
    biash = np.ascontiguousarray(b_gate.astype(np.float32).reshape(NOB, 128).T)
    return wdr, invd, biash


def make_in_maps(inputs: np.ndarray, W_gate: np.ndarray, b_gate: np.ndarray):
    wdr, invd, biash = prep_shared(W_gate, b_gate)
    in_maps = []
    for c in range(B):
        xT_c = np.ascontiguousarray(inputs[c].T).reshape(NJ, 128, L)
        in_maps.append({"xT": xT_c, "wdr": wdr, "invd": invd, "biash": biash})
    return in_maps


def kernel(inputs: np.ndarray, W_gate: np.ndarray, b_gate: np.ndarray, **run_kwargs):
    inputs = np.asarray(inputs, dtype=np.float32)
    W_gate = np.asarray(W_gate, dtype=np.float32)
    b_gate = np.asarray(b_gate, dtype=np.float32)
    assert inputs.shape == (B, L, D)

    in_maps = make_in_maps(inputs, W_gate, b_gate)
    nc = build_nc()
    res = bass_utils.run_bass_kernel_spmd(
        nc, in_maps, core_ids=list(range(B)), **run_kwargs
    )

    gating = np.empty((B, L, D), dtype=np.float32)
    average = np.empty((B, L, D), dtype=np.float32)
    for c in range(B):
        gating[c] = res.results[c]["gatT"].reshape(D, L).T
        average[c] = res.results[c]["avgT"].reshape(D, L).T
    if run_kwargs:
        _CACHE["last_results"] = res
    return gating, average
